# revision 4
# baseline (speedup 1.0000x reference)
"""Trainium2 Bass kernel for nn_CGM (context-gated modulation).

Math (per batch element b):
    att[c,k]  = sum_hw feature[c,hw] * map[k,hw]          # [C,K] contraction
    scale[c]  = 1 + sum_k sigmoid(att[c,k]) * gamma[k]
    out[c,hw] = feature[c,hw] * scale[c]

Sharding: pure data parallel — one batch element per NeuronCore (B=8).

Device dataflow per core:
  - feature [256, 16384] f32 loaded natural (C on partitions) in 16 chunk
    tiles; kept resident in SBUF for the final scaling (no second read).
  - map is transposed on the host into per-hw-block [128, 19] stationary
    tiles (tiny: 1.2 MiB).
  - Per 128-wide hw block: PE transpose (fp32r transpose mode, exact) of
    both c-halves -> PSUM [128, 256] -> DVE copy to SBUF -> PE matmul
    (float32r) accumulating att^T [19, 256] in PSUM over all 128 blocks.
  - sigmoid(att^T) on ACT into X[0:19]; X row 19 = ones; then
    scale' = X^T @ [gamma; 1] on PE gives per-partition [128,1] factors
    (the appended ones row folds in the "+1").
  - ACT per-partition multiply rescales the resident feature tiles in
    place; chunks stream back to DRAM.
"""

import numpy as np
from contextlib import ExitStack

import concourse.bacc as bacc
import concourse.tile as tile
import concourse.mybir as mybir
from concourse import bass_utils

B, C, K = 8, 256, 19
H = W = 128
HW = H * W            # 16384
P = 128               # SBUF partitions
NB = HW // P          # 128 hw blocks
CH = 2048             # chunk width (f32 -> [128, 2048] = 1 MiB per DMA)
NCH = HW // CH        # 8 chunks per c-half
BPC = CH // P         # hw blocks per chunk

# dtype used for the att matmul operands (f32r: full-rate PE, near-fp32)
MM_DT = mybir.dt.float32r
F32 = mybir.dt.float32

_prog_cache = {}


def _build_program():
    nc = bacc.Bacc("TRN2", target_bir_lowering=False, debug=False)

    feat = nc.dram_tensor("feature", [C, HW], F32, kind="ExternalInput")
    mapt = nc.dram_tensor("mapt", [P, NB * K], F32, kind="ExternalInput")
    gma = nc.dram_tensor("gma", [K + 1, 1], F32, kind="ExternalInput")
    idn = nc.dram_tensor("idn", [P, P], F32, kind="ExternalInput")
    out_d = nc.dram_tensor("out", [C, HW], F32, kind="ExternalOutput")

    with tile.TileContext(nc) as tc, ExitStack() as ctx:
        sb = ctx.enter_context(tc.tile_pool(name="sb", bufs=1))
        fts_pool = ctx.enter_context(tc.tile_pool(name="fts", bufs=4))
        ps = ctx.enter_context(tc.tile_pool(name="ps", bufs=1, space="PSUM"))
        ftp_pool = ctx.enter_context(
            tc.tile_pool(name="ftp", bufs=3, space="PSUM")
        )

        ident = sb.tile([P, P], F32, name="ident")
        nc.sync.dma_start(ident[:], idn[:])
        mT = sb.tile([P, NB * K], F32, name="mT")
        nc.sync.dma_start(mT[:], mapt[:])
        # Walrus requires fp32r matmul operands to be *produced* as fp32r
        # (rounded); a bitcast of DMA-written f32 is rejected. One DVE cast
        # copy rounds the map once.
        mTr = sb.tile([P, NB * K], MM_DT, name="mTr")
        nc.vector.tensor_copy(mTr[:], mT[:])
        gA = sb.tile([K + 1, 1], F32, name="gA")
        nc.sync.dma_start(gA[:], gma[:])
        # Row K stays 1.0 (folds the "+1" into the gamma matmul); the
        # sigmoid later overwrites rows 0..K-1. Whole-tile memset because
        # engine writes must start at partition 0/32/64/96.
        X = sb.tile([K + 1, C], F32, name="X")
        nc.vector.memset(X[:], 1.0)

        # Load feature chunks, interleaving the two c-halves so early hw
        # blocks have both halves available as soon as possible.
        F = [[None] * NCH for _ in range(2)]
        for j in range(NCH):
            for h in range(2):
                t = sb.tile([P, CH], F32, name=f"F{h}_{j}", tag=f"F{h}_{j}")
                nc.sync.dma_start(
                    t[:], feat[h * P : (h + 1) * P, j * CH : (j + 1) * CH]
                )
                F[h][j] = t

        attT = ps.tile([K, C], F32, name="attT")

        # Software-pipelined by 2: transpose block i, copy block i-1,
        # matmul block i-2 — keeps PE from stalling on the DVE copy.
        ftps, ftss = {}, {}
        for i in range(NB + 2):
            if i < NB:
                j, o = divmod(i * P, CH)
                ftp = ftp_pool.tile([P, C], F32, name="ftp", tag="ftp")
                for h in range(2):
                    nc.tensor.transpose(
                        ftp[:, h * P : (h + 1) * P],
                        F[h][j][:, o : o + P],
                        ident[:],
                    )
                ftps[i] = ftp
            if 0 <= i - 1 < NB:
                # DVE cast copy rounds to fp32r while moving PSUM->SBUF.
                fts = fts_pool.tile([P, C], MM_DT, name="fts", tag="fts")
                nc.vector.tensor_copy(fts[:], ftps.pop(i - 1)[:])
                ftss[i - 1] = fts
            if 0 <= i - 2 < NB:
                ii = i - 2
                fts = ftss.pop(ii)
                nc.tensor.matmul(
                    attT[:],
                    mTr[:, ii * K : (ii + 1) * K],
                    fts[:],
                    start=(ii == 0),
                    stop=(ii == NB - 1),
                )

        nc.scalar.activation(
            X[0:K, :], attT[:], mybir.ActivationFunctionType.Sigmoid
        )
        scale_sb = sb.tile([P, 2], F32, name="scale_sb")
        for h in range(2):
            sp = ps.tile([P, 1], F32, name=f"sp{h}", tag=f"sp{h}")
            nc.tensor.matmul(
                sp[:], X[:, h * P : (h + 1) * P], gA[:], start=True, stop=True
            )
            nc.vector.tensor_copy(scale_sb[:, h : h + 1], sp[:])

        for h in range(2):
            for j in range(NCH):
                t = F[h][j]
                nc.scalar.mul(t[:], t[:], scale_sb[:, h : h + 1])
                nc.sync.dma_start(
                    out_d[h * P : (h + 1) * P, j * CH : (j + 1) * CH], t[:]
                )

    nc.compile()
    return nc


def get_program():
    if "nc" not in _prog_cache:
        _prog_cache["nc"] = _build_program()
    return _prog_cache["nc"]


def make_in_maps(feature, map, gamma):
    """Host-side sharding + layout prep. feature [B,C,H,W], map [B,K,H,W],
    gamma [1,1,1,1,K] -> one in_map per core."""
    feature = np.asarray(feature, dtype=np.float32)
    map = np.asarray(map, dtype=np.float32)
    gamma = np.asarray(gamma, dtype=np.float32)

    gma = np.concatenate(
        [gamma.reshape(K), np.ones((1,), np.float32)]
    ).reshape(K + 1, 1)
    idn = np.eye(P, dtype=np.float32)

    in_maps = []
    for b in range(B):
        f_b = np.ascontiguousarray(feature[b].reshape(C, HW))
        # mapt[p, n*K + k] = map[b, k, n*128 + p]
        m_b = (
            map[b]
            .reshape(K, NB, P)
            .transpose(2, 1, 0)  # [p, n, k]
            .reshape(P, NB * K)
        )
        in_maps.append(
            {
                "feature": f_b,
                "mapt": np.ascontiguousarray(m_b),
                "gma": gma,
                "idn": idn,
            }
        )
    return in_maps


def run(inputs, trace=False, **kwargs):
    nc = get_program()
    in_maps = make_in_maps(inputs["feature"], inputs["map"], inputs["gamma"])
    res = bass_utils.run_bass_kernel_spmd(
        nc, in_maps, core_ids=list(range(B)), trace=trace, **kwargs
    )
    out = np.empty((B, C, H, W), dtype=np.float32)
    for b in range(B):
        out[b] = res.results[b]["out"].reshape(C, H, W)
    return out, res


def kernel(**inputs):
    out, _ = run(inputs)
    return out


if __name__ == "__main__":
    rng = np.random.default_rng(0)
    inputs = {
        "feature": rng.standard_normal((B, C, H, W), dtype=np.float32),
        "map": rng.random((B, K, H, W), dtype=np.float32),
        "gamma": (rng.standard_normal((1, 1, 1, 1, K)) * 0.1).astype(
            np.float32
        ),
    }
    out = kernel(**inputs)
    print("out", out.shape, out.dtype)
